# revision 10
# baseline (speedup 1.0000x reference)
"""Trainium2 Bass kernel for CharOffsetRoPEAttention.

Full-input contract: kernel(**inputs) takes the complete tensors
(x [4,2048,1024], wq/wk/wv/wo [1024,1024], position_ids [4,2048]) and
returns the full output [4,2048,1024].

Sharding: batch x head-group over 8 cores. Core c handles batch c//2 and
heads [8*(c%2), 8*(c%2)+8). q/k/v weights are split column-wise (by head),
wo row-wise; each core computes a partial output for its batch that the
host sums pairwise.

Single fused phase, engine-dieted so the scalar engine (ACT) does nothing
but the softmax exp (the serial floor: T*T*8 exps/core ~ 266us at 1.2GHz):
  - projections (f32r matmuls) and RoPE (DVE/Pool, bf16) are emitted
    interleaved with the attention stream so PE/DVE/Pool work hides under
    the ACT-bound exp stream.
  - scores per (head, 512-query strip): two j-tiles share a [128,1024]
    PSUM pair, one batched exp [128,1024] -> et bf16.
  - attn@v runs TRANSPOSED: out[i, hd+1] = et.T @ v, N=65 per matmul
    (half the PE cycles of the [hd+1, i] orientation), and the softmax
    denominator lands as a per-partition scalar -> normalization is a
    plain DVE reciprocal + tensor_scalar_mul, no cross-partition
    broadcast at all.
  - a PE transpose (identity matmul) flips normalized [i, d] tiles back
    to [d, i] bf16 for the wo projection (bf16 x bf16 matmuls).

RoPE uses a host-side row permutation of wq/wk (per head: evens then
odds) so the rotation is an elementwise mul plus four [32,*] swapped
quarter-muls with sign-baked sin rows (all bf16 on DVE/Pool).
"""

import os
from contextlib import ExitStack

import numpy as np

import concourse.bass as bass
import concourse.mybir as mybir
import concourse.tile as tile
from concourse import bacc
from concourse.bass_utils import run_bass_kernel_spmd

B, T, D, H, HD = 4, 2048, 1024, 16, 64
NCORES = 8
HG = 2            # head groups (cores per batch)
HLOC = H // HG    # 8 heads per core
DLOC = HLOC * HD  # 512 local dims per core
KT = D // 128     # 8 k-tiles for the projections
NB = 4            # 512-col t-chunks
THETA = 10000.0

F32 = mybir.dt.float32
F32R = mybir.dt.float32r
BF16 = mybir.dt.bfloat16
EXP = mybir.ActivationFunctionType.Exp


def build_program(loop_n: int = 1):
    nc = bacc.Bacc()

    xT_d = nc.declare_dram_parameter("xT", [D, T], BF16, isOutput=False)
    wqT_d = nc.declare_dram_parameter("wqT", [D, DLOC], BF16, isOutput=False)
    wkT_d = nc.declare_dram_parameter("wkT", [D, DLOC], BF16, isOutput=False)
    wvT_d = nc.declare_dram_parameter("wvT", [D, DLOC], BF16, isOutput=False)
    woT_d = nc.declare_dram_parameter("woT", [DLOC, D], BF16, isOutput=False)
    cos_d = nc.declare_dram_parameter("cosT", [128, T], BF16, isOutput=False)
    sin_d = nc.declare_dram_parameter("sinT", [128, T], BF16, isOutput=False)
    id_d = nc.declare_dram_parameter("ident", [128, 128], BF16, isOutput=False)
    out_d = nc.declare_dram_parameter("out", [T, D], F32, isOutput=True)

    with tile.TileContext(nc) as tc, ExitStack() as top:
        if loop_n > 1:
            top.enter_context(tc.For_i(0, loop_n, 1))
        # ---------------- persistent SBUF ----------------
        pool_qk = top.enter_context(tc.tile_pool(name="qk", bufs=1))
        pool_v = top.enter_context(tc.tile_pool(name="vp", bufs=1))
        pool_w = top.enter_context(tc.tile_pool(name="wt", bufs=1))
        pool_wo = top.enter_context(tc.tile_pool(name="wop", bufs=1))
        pool_cs = top.enter_context(tc.tile_pool(name="cs", bufs=1))
        pool_ao = top.enter_context(tc.tile_pool(name="ao", bufs=1))
        pool_x = top.enter_context(tc.tile_pool(name="xsl", bufs=12))
        pool_rt = top.enter_context(tc.tile_pool(name="rt", bufs=6))
        pool_et = top.enter_context(tc.tile_pool(name="ep", bufs=30))
        pool_aot = top.enter_context(tc.tile_pool(name="aots", bufs=2))
        pool_rz = top.enter_context(tc.tile_pool(name="rz", bufs=4))
        pool_st = top.enter_context(tc.tile_pool(name="stp", bufs=4))
        # ---------------- PSUM ----------------
        pool_sp = top.enter_context(tc.tile_pool(name="sp", bufs=2, space="PSUM"))
        pool_oT = top.enter_context(tc.tile_pool(name="oT", bufs=1, space="PSUM"))
        pool_pT = top.enter_context(tc.tile_pool(name="pT", bufs=1, space="PSUM"))
        pool_ut = top.enter_context(tc.tile_pool(name="ut", bufs=2, space="PSUM"))

        qT_t = [pool_qk.tile([128, T], BF16, name=f"qT{m}") for m in range(4)]
        kT_t = [pool_qk.tile([128, T], BF16, name=f"kT{m}") for m in range(4)]
        v_t = [pool_v.tile([128, HLOC, HD + 1], BF16, name=f"v{i}") for i in range(16)]
        ao_t = [pool_ao.tile([128, T], BF16, name=f"ao{m}") for m in range(4)]
        ident = pool_cs.tile([128, 128], BF16, name="ident")
        cos_sb = pool_cs.tile([128, T], BF16, name="cos_sb")
        sin_sb = pool_cs.tile([128, T], BF16, name="sin_sb")
        ones8 = pool_cs.tile([128, HLOC], BF16, name="ones8")

        # ---------------- DMA: most-urgent first ----------------
        nc.sync.dma_start(out=cos_sb[:], in_=cos_d[:])
        nc.sync.dma_start(out=sin_sb[:], in_=sin_d[:])
        nc.sync.dma_start(out=ident[:], in_=id_d[:])
        w_tiles = {}
        for nm, dram in (("q", wqT_d), ("k", wkT_d), ("v", wvT_d)):
            w_tiles[nm] = [
                pool_w.tile([128, DLOC], BF16, name=f"w{nm}{k}") for k in range(KT)
            ]
        for k in range(KT):
            nc.sync.dma_start(out=w_tiles["q"][k][:], in_=wqT_d[k * 128 : (k + 1) * 128, :])
        for k in range(KT):
            nc.sync.dma_start(out=w_tiles["k"][k][:], in_=wkT_d[k * 128 : (k + 1) * 128, :])
        x_sl = {}  # (nb, k) -> tile
        def load_x(nb):
            for k in range(KT):
                xs = pool_x.tile([128, 512], BF16, name="x_sl", tag="x_sl")
                nc.sync.dma_start(
                    out=xs[:],
                    in_=xT_d[k * 128 : (k + 1) * 128, nb * 512 : (nb + 1) * 512],
                )
                x_sl[(nb, k)] = xs

        load_x(0)
        for k in range(KT):
            nc.sync.dma_start(out=w_tiles["v"][k][:], in_=wvT_d[k * 128 : (k + 1) * 128, :])
        load_x(1)
        woT_t = [pool_wo.tile([128, D], BF16, name=f"wo{k}") for k in range(4)]
        for k in range(4):
            nc.sync.dma_start(out=woT_t[k][:], in_=woT_d[k * 128 : (k + 1) * 128, :])
        load_x(2)
        load_x(3)
        # steady-state q projections re-fetch x chunks (cheap: DMA is idle
        # then; keeping all four chunks live would blow SBUF)

        nc.gpsimd.memset(ones8[:], 1.0)
        for i in range(16):
            nc.vector.tensor_copy(v_t[i][:, :, HD : HD + 1], ones8[:].unsqueeze(2))

        # ---------------- building blocks ----------------
        def proj_qk(nm, dst, m, nb):
            """Project 128 dims (tile m) of q or k for t-chunk nb, with RoPE."""
            ns = slice(nb * 512, (nb + 1) * 512)
            pq = pool_ut.tile([128, 512], F32, name="pq", tag="ut")
            for k in range(KT):
                nc.tensor.matmul(
                    pq[:],
                    lhsT=w_tiles[nm][k][:, m * 128 : (m + 1) * 128],
                    rhs=x_sl[(nb, k)][:],
                    start=(k == 0),
                    stop=(k == KT - 1),
                )
            nc.vector.tensor_copy(dst[m][:, ns], pq[:])
            # RoPE in place: rows per 64-block are [r(32); i(32)].
            # q_rot = q*cos + swap32(q)*sin_signed (sin rows carry
            # [+s; -s; +s; -s]). DVE requires equal input base partitions,
            # so each quarter mul reads aligned inputs and writes the
            # swapped-out quarter.
            tcos = pool_rt.tile([128, 512], BF16, name="tcos", tag="rt")
            tsw = pool_rt.tile([128, 512], BF16, name="tsw", tag="rt")
            nc.gpsimd.tensor_mul(tcos[:], dst[m][:, ns], cos_sb[:, ns])
            for q0 in (0, 32, 64, 96):
                src = q0 ^ 32
                nc.vector.tensor_mul(
                    tsw[q0 : q0 + 32, :],
                    dst[m][src : src + 32, ns],
                    sin_sb[src : src + 32, ns],
                )
            nc.gpsimd.tensor_add(dst[m][:, ns], tcos[:], tsw[:])

        def proj_v(tt):
            """Project v for t-block tt (128 rows of t) -> v_t[tt]."""
            nb, sub = tt // 4, tt % 4
            pv = pool_ut.tile([128, 512], F32, name="pv", tag="ut")
            for k in range(KT):
                nc.tensor.matmul(
                    pv[:],
                    lhsT=x_sl[(nb, k)][:, sub * 128 : (sub + 1) * 128],
                    rhs=w_tiles["v"][k][:],
                    start=(k == 0),
                    stop=(k == KT - 1),
                )
            nc.vector.tensor_copy(
                v_t[tt][:, :, 0:HD],
                pv[:].rearrange("p (h e) -> p h e", h=HLOC),
            )

        # per-unit state: unit = (h, I): full attention for head h,
        # query strip I (512 queries)
        unit_et = {}   # (h, I) -> list of 8 et tiles [128, 2, 512]
        unit_oT = {}   # (h, I) -> psum accumulator [128, 4, 65]

        def unit_scores(h, I, jps):
            """Score+exp for j-pairs jps of unit (h, I)."""
            g, half = h // 2, 64 * (h % 2)
            qm, km = qT_t[g], kT_t[g]
            ets = unit_et.setdefault((h, I), {})
            iss = slice(I * 512, (I + 1) * 512)
            for jp in jps:
                sp = pool_sp.tile([128, 1024], F32, name="s_ps", tag="sp")
                for r in range(2):
                    j = 2 * jp + r
                    nc.tensor.matmul(
                        sp[:, r * 512 : (r + 1) * 512],
                        lhsT=km[half : half + 64, j * 128 : (j + 1) * 128],
                        rhs=qm[half : half + 64, iss],
                        start=True,
                        stop=True,
                        tile_position=(half, 0),
                    )
                et = pool_et.tile([128, 2, 512], BF16, name="e_t", tag="e")
                nc.scalar.activation(
                    out=et[:].rearrange("p a b -> p (a b)"),
                    in_=sp[:],
                    func=EXP,
                    scale=0.125,
                )
                ets[jp] = et

        def unit_avs(h, I):
            """Transposed attn@v accumulation over all 16 j-tiles.

            PSUM accumulation groups must be contiguous per region, so the
            ib sub-tiles are the OUTER loop (each [128,65] region sees its
            16 matmuls back-to-back)."""
            key = (h, I)
            oT = pool_oT.tile([128, 4, HD + 1], F32, name="oT", tag="oT")
            unit_oT[key] = oT
            ets = unit_et[key]
            for ib in range(4):
                for jp in range(8):
                    et = ets[jp]
                    for r in range(2):
                        j = 2 * jp + r
                        nc.tensor.matmul(
                            oT[:, ib, :],
                            lhsT=et[:, r, ib * 128 : (ib + 1) * 128],
                            rhs=v_t[j][:, h, :],
                            start=(jp == 0 and r == 0),
                            stop=(jp == 7 and r == 1),
                        )

        def unit_tail(h, I):
            """Normalize, transpose back to [d, t], store into ao_t."""
            g, half = h // 2, 64 * (h % 2)
            oT = unit_oT.pop((h, I))
            unit_et.pop((h, I))
            aos = pool_aot.tile([128, 4, HD], BF16, name="aos", tag="aos")
            for ib in range(4):
                rz = pool_rz.tile([128, 1], F32, name="rz", tag="rz")
                nc.vector.reciprocal(rz[:], oT[:, ib, HD : HD + 1])
                nc.vector.tensor_scalar_mul(aos[:, ib, :], oT[:, ib, 0:HD], rz[:])
            pT = pool_pT.tile([HD, 512], BF16, name="pT", tag="pT")
            for ib in range(4):
                nc.tensor.transpose(
                    pT[:, ib * 128 : (ib + 1) * 128], aos[:, ib, :], ident[:]
                )
            nc.vector.tensor_copy(
                ao_t[g][half : half + HD, I * 512 : (I + 1) * 512], pT[:]
            )

        def wo_block(tt, nblk):
            po = pool_ut.tile([128, 512], F32, name="po", tag="ut")
            for k in range(4):
                nc.tensor.matmul(
                    po[:],
                    lhsT=ao_t[k][:, tt * 128 : (tt + 1) * 128],
                    rhs=woT_t[k][:, nblk * 512 : (nblk + 1) * 512],
                    start=(k == 0),
                    stop=(k == 3),
                )
            st = pool_st.tile([128, 512], F32, name="st", tag="st")
            nc.vector.tensor_copy(st[:], po[:])
            nc.sync.dma_start(
                out=out_d[tt * 128 : (tt + 1) * 128, nblk * 512 : (nblk + 1) * 512],
                in_=st[:],
            )

        # ---------------- emission schedule ----------------
        # 1) q-proj chunk 0 first (unit (0,0) scores need it), then per
        #    t-chunk: k-proj + v-proj, with unit (0,0)/(1,0) partial
        #    scores+exps+avs interleaved so ACT ramps as j-coverage grows.
        for m in range(4):
            proj_qk("q", qT_t, m, 0)
        for nb in range(NB):
            for m in range(4):
                proj_qk("k", kT_t, m, nb)
            if nb < 3:
                for sub in range(4):
                    proj_v(nb * 4 + sub)
            unit_scores(0, 0, [2 * nb, 2 * nb + 1])
            for lag in (1, 2, 3):
                if nb >= lag:
                    unit_scores(lag, 0, [2 * (nb - lag), 2 * (nb - lag) + 1])
        # finish: last v chunk, staggered score tails, and two more
        # pre-scored units so ACT has a backlog entering steady state
        unit_scores(1, 0, [6, 7])
        for sub in range(4):
            proj_v(12 + sub)
        unit_scores(2, 0, [4, 5])
        unit_scores(3, 0, [2, 3])
        unit_scores(2, 0, [6, 7])
        unit_scores(4, 0, [0, 1, 2, 3])
        unit_scores(3, 0, [4, 5, 6, 7])
        unit_scores(4, 0, [4, 5, 6, 7])

        # 2) steady state: one-unit-deep software pipeline (scores of the
        #    next unit run ahead of this unit's avs); qproj chunk I+1 and
        #    strip I's wo blocks spread between units.
        units = [(h, I) for I in range(4) for h in range(HLOC)]
        scored = {(h, 0) for h in range(5)}
        avsed = set()
        wo_pending = []
        for n, u in enumerate(units):
            h, I = u
            if u not in scored:
                unit_scores(*u, range(8))
                scored.add(u)
            if n + 1 < len(units) and units[n + 1] not in scored:
                unit_scores(*units[n + 1], range(8))
                scored.add(units[n + 1])
            if u not in avsed:
                unit_avs(h, I)
                avsed.add(u)
            unit_tail(h, I)
            if I < 3 and h < 4:
                if h == 0:
                    load_x(I + 1)  # re-fetch x chunk for next strip's qproj
                proj_qk("q", qT_t, h, I + 1)
            if wo_pending:
                wo_block(*wo_pending.pop(0))
            if h == HLOC - 1:
                # strip I's ao is complete -> queue its wo blocks
                for tt in range(I * 4, (I + 1) * 4):
                    for nblk in range(2):
                        wo_pending.append((tt, nblk))
        while wo_pending:
            wo_block(*wo_pending.pop(0))

    nc.finalize()
    return nc


def _host_tables(pos_row):
    inv_freq = (
        1.0
        / (np.float32(THETA) ** (np.arange(0, HD, 2, dtype=np.float32) / np.float32(HD)))
    ).astype(np.float32)
    ang = (pos_row.astype(np.float32)[:, None] * inv_freq[None, :]).astype(np.float32)
    cos32 = np.cos(ang).astype(np.float32).T  # [32, T]
    sin32 = np.sin(ang).astype(np.float32).T
    cosT = np.tile(cos32, (4, 1))
    sinT = np.concatenate([sin32, -sin32, sin32, -sin32], axis=0)
    return cosT, sinT


def prep_in_maps(x, wq, wk, wv, wo, position_ids):
    """Host-side sharding + layout prep (pure numpy)."""
    import ml_dtypes

    bf = ml_dtypes.bfloat16
    x = np.asarray(x, dtype=np.float32)
    wq = np.asarray(wq, dtype=np.float32)
    wk = np.asarray(wk, dtype=np.float32)
    wv = np.asarray(wv, dtype=np.float32)
    wo = np.asarray(wo, dtype=np.float32)
    pos = np.asarray(position_ids)

    # per-head rotate-half permutation: evens then odds
    base = np.concatenate([np.arange(0, HD, 2), np.arange(1, HD, 2)])
    perm = np.concatenate([h * HD + base for h in range(HLOC)])
    ident = np.eye(128, dtype=np.float32)

    in_maps = []
    for c in range(NCORES):
        b, hg = c // HG, c % HG
        rows = slice(hg * DLOC, (hg + 1) * DLOC)
        cosT, sinT = _host_tables(pos[b])
        in_maps.append(
            {
                "xT": np.ascontiguousarray(x[b].T).astype(bf),
                "wqT": np.ascontiguousarray(wq[rows, :][perm].T).astype(bf),
                "wkT": np.ascontiguousarray(wk[rows, :][perm].T).astype(bf),
                "wvT": np.ascontiguousarray(wv[rows, :].T).astype(bf),
                "woT": np.ascontiguousarray(wo[:, rows].T).astype(bf),
                "cosT": np.ascontiguousarray(cosT).astype(bf),
                "sinT": np.ascontiguousarray(sinT).astype(bf),
                "ident": ident.astype(bf),
            }
        )
    return in_maps


def gather(results):
    """Sum the per-core partial outputs pairwise into the full output."""
    out = np.empty((B, T, D), dtype=np.float32)
    for b in range(B):
        out[b] = results[2 * b]["out"] + results[2 * b + 1]["out"]
    return out


_CACHED_NC = None


def kernel(x, wq, wk, wv, wo, position_ids):
    global _CACHED_NC
    if _CACHED_NC is None:
        _CACHED_NC = build_program()
    in_maps = prep_in_maps(x, wq, wk, wv, wo, position_ids)
    res = run_bass_kernel_spmd(_CACHED_NC, in_maps, list(range(NCORES)))
    return gather(res.results)
